# revision 16
# baseline (speedup 1.0000x reference)
"""Trainium2 Bass kernel for a GQA attention layer (B=2, S=2048, D=4096,
32 q-heads, 8 kv-heads, HD=128, RoPE, causal mask).

Sharding: 8 cores = 2 (batch) x 4 (head groups). Each core handles one
batch and 8 q-heads / 2 kv-heads: column-parallel wq/wk/wv, row-parallel
wo. Each core emits a partial [S, D] output; the host sums the 4 partials
per batch. No collectives.

Device dataflow (per core):
  phase 1: QKV projections from host-pretransposed xT (feature-major),
           RoPE applied in a "split" head layout (host permutes wq/wk
           columns so real/imag parts land in partition halves; the
           cross-partition swap is an SBUF->SBUF DMA).
  phase 2: scoresT[sk,sq] = K^T-tiles (stationary) x Q^T (moving); exp on
           ScalarE with scale=1/sqrt(HD); causal handling = skip fully
           masked sk-tiles + one [128,128] mask-tile add on the diagonal;
           softmax denominator via an all-ones stationary matmul that
           broadcasts the per-query sum to all 128 psum partitions;
           attnT accumulated with V (token-major) stationary; 1/denom
           applied during psum evacuation (DVE reciprocal + mul). The
           denom/attn matmuls trail the scores/exp stream by 3 tiles so
           the PE never waits on the ScalarE exp.
  phase 3: out_partial = attnT^T x wo-rows, streamed per 512-col block.
           Projection panels, attention blocks and the output projection
           are software-pipelined (panel n+1 overlaps attention block n).
"""

import sys

if "/opt/trn_rl_repo" not in sys.path:
    sys.path.insert(0, "/opt/trn_rl_repo")

import math
from contextlib import ExitStack

import ml_dtypes
import numpy as np

import concourse.bass as bass  # noqa: F401  (AP types used implicitly)
import concourse.tile as tile
from concourse import bacc, mybir
from concourse.bass_utils import run_bass_kernel_spmd

BF16 = ml_dtypes.bfloat16
F32 = mybir.dt.float32
BF = mybir.dt.bfloat16

B, S, D = 2, 2048, 4096
NH, NKV, HD = 32, 8, 128
G = 4  # head groups -> cores per batch
HPG = NH // G  # 8 q heads per core
KPG = NKV // G  # 2 kv heads per core
SCALE = 1.0 / math.sqrt(HD)

NFT = D // 128  # 32 feature tiles (contraction)
PTOK = 512  # token panel width in phase 1
NPANEL = S // PTOK  # 4
NTT = S // 128  # 16 token tiles
NSQ = S // 512  # 4 sq tiles
NOD = D // 512  # 8 out-D tiles

_CACHE = {}


def _build_program(phases=(1, 2, 3), reps=1):
    nc = bacc.Bacc("TRN2", target_bir_lowering=False, debug=False, num_devices=8)

    xt = nc.dram_tensor("xt", [D, S], BF, kind="ExternalInput").ap()
    wq = nc.dram_tensor("wq", [HPG, 128, NFT * 128], BF, kind="ExternalInput").ap()
    wk = nc.dram_tensor("wk", [KPG, 128, NFT * 128], BF, kind="ExternalInput").ap()
    wv = nc.dram_tensor("wv", [128, NFT * KPG * 128], BF, kind="ExternalInput").ap()
    wo = nc.dram_tensor("wo", [NOD, 128, HPG * 512], BF, kind="ExternalInput").ap()
    cosb = nc.dram_tensor("cosb", [128, S], BF, kind="ExternalInput").ap()
    sinb = nc.dram_tensor("sinb", [128, S], BF, kind="ExternalInput").ap()
    trim = nc.dram_tensor("trim", [128, 128], BF, kind="ExternalInput").ap()
    ones32 = nc.dram_tensor("ones32", [128, 128], F32, kind="ExternalInput").ap()
    outp = nc.dram_tensor("outp", [S, D], F32, kind="ExternalOutput").ap()

    EXP = mybir.ActivationFunctionType.Exp
    MULT = mybir.AluOpType.mult
    F32R = mybir.dt.float32r

    with tile.TileContext(nc) as tc, ExitStack() as ctx:
        pool = lambda name, bufs: ctx.enter_context(tc.tile_pool(name=name, bufs=bufs))
        ppool = lambda name, bufs: ctx.enter_context(
            tc.tile_pool(name=name, bufs=bufs, space="PSUM")
        )

        persist = pool("persist", 1)
        xpool = pool("xpool", 4)
        wqpool = pool("wqpool", 3)
        ropepool = pool("ropepool", 2)
        probpool = pool("probpool", 4)
        bigden = pool("bigden", 2)
        wopool = pool("wopool", 2)
        outpool = pool("outpool", 4)

        psS = ppool("psS", 2)  # [128,1024] f32 (2 banks): score pair-groups
        psGP = ppool("psGP", 2)  # [128,512] f32: QKV proj, O-proj, denom bcast
        psAt = ppool("psAt", 2)  # [128,512] f32: attn accum

        # ---- persistent tiles ----
        qt = [persist.tile([128, S], BF, tag=f"qt{h}", name=f"qt{h}") for h in range(HPG)]
        kt = [persist.tile([128, S], BF, tag=f"kt{k}", name=f"kt{k}") for k in range(KPG)]
        v_sb = persist.tile([128, NTT * KPG * 128], BF, tag="v", name="v_sb")
        v_w_sb = persist.tile([128, NFT * KPG * 128], BF, tag="vw", name="v_w_sb")
        at = [persist.tile([128, S], BF, tag=f"at{h}", name=f"at{h}") for h in range(HPG)]
        cos_sb = persist.tile([128, S], BF, tag="cos", name="cos_sb")
        sin_sb = persist.tile([128, S], BF, tag="sin", name="sin_sb")
        tri_sb = persist.tile([128, 128], BF, tag="tri", name="tri_sb")
        ones_f32 = persist.tile([128, 128], F32, tag="ones32", name="ones_f32")
        ones_sb = persist.tile([128, 128], F32R, tag="ones", name="ones_sb")

        nc.sync.dma_start(cos_sb[:], cosb[:])
        nc.sync.dma_start(sin_sb[:], sinb[:])
        nc.sync.dma_start(tri_sb[:], trim[:])
        nc.sync.dma_start(ones_f32[:], ones32[:])
        nc.vector.tensor_copy(ones_sb[:], ones_f32[:])

        # ======== pipelined: proj panel n+1 overlaps attention block n ====
        do1, do2, do3 = (1 in phases), (2 in phases), (3 in phases)
        xt_v = xt.rearrange("(f p) t -> p f t", p=128)
        qk_dst = list(qt) + list(kt)

        def proj_panel(n, pre=None):
            tok0 = n * PTOK
            if pre is not None:
                halves = pre(n)
            else:
                halves = []
                for q4 in range(4):
                    xq = xpool.tile([128, 8 * PTOK], BF, tag="xts", name="xq")
                    nc.sync.dma_start(
                        xq.rearrange("p (f t) -> p f t", t=PTOK),
                        xt_v[:, q4 * 8 : (q4 + 1) * 8, tok0 : tok0 + PTOK],
                    )
                    halves.append((xq, q4 * 8))

            for hh in range(HPG + KPG):
                wsrc = wq[hh] if hh < HPG else wk[hh - HPG]
                wh = wqpool.tile([128, NFT * 128], BF, tag="wqt", name="wh")
                nc.sync.dma_start(wh[:], wsrc)
                ps = psGP.tile([128, PTOK], F32, tag="psGP", name="ps_qk")
                for xtile, f0 in halves:
                    for fl in range(8):
                        f = f0 + fl
                        nc.tensor.matmul(
                            ps[:],
                            wh[:, f * 128 : (f + 1) * 128],
                            xtile[:, fl * PTOK : (fl + 1) * PTOK],
                            start=(f == 0),
                            stop=(f == NFT - 1),
                        )
                nc.vector.tensor_copy(qk_dst[hh][:, tok0 : tok0 + PTOK], ps[:])

            # V proj: two 128-token chains share one [128,512] psum tile
            for mp in range(PTOK // 256):
                ps = psGP.tile([128, PTOK], F32, tag="psGP", name="ps_v")
                for half in range(2):
                    m = mp * 2 + half
                    for xtile, f0 in halves:
                        for fl in range(8):
                            f = f0 + fl
                            nc.tensor.matmul(
                                ps[:, half * 256 : half * 256 + 256],
                                xtile[:, fl * PTOK + m * 128 : fl * PTOK + m * 128 + 128],
                                v_w_sb[:, f * 256 : (f + 1) * 256],
                                start=(f == 0),
                                stop=(f == NFT - 1),
                            )
                tglob = n * (PTOK // 256) + mp
                nc.vector.tensor_copy(v_sb[:, tglob * 512 : (tglob + 1) * 512], ps[:])

            # RoPE this panel: dst = dst*C + swap_halves(dst)*S2 (K first)
            for dst in list(kt) + list(qt):
                rsw = ropepool.tile([128, PTOK], BF, tag="rsw", name="rsw")
                nc.sync.dma_start(rsw[0:64, :], dst[64:128, tok0 : tok0 + PTOK])
                nc.sync.dma_start(rsw[64:128, :], dst[0:64, tok0 : tok0 + PTOK])
                nc.vector.tensor_mul(rsw[:], rsw[:], sin_sb[:, tok0 : tok0 + PTOK])
                nc.vector.tensor_mul(
                    dst[:, tok0 : tok0 + PTOK],
                    dst[:, tok0 : tok0 + PTOK],
                    cos_sb[:, tok0 : tok0 + PTOK],
                )
                nc.vector.tensor_add(
                    dst[:, tok0 : tok0 + PTOK],
                    dst[:, tok0 : tok0 + PTOK],
                    rsw[:],
                )

        def attn_block(j):
            sq0 = j * 512
            n_sk = 4 * (j + 1)
            n_grp = n_sk // 2
            SKEW = 2  # attn/den for group g trail the scores of group g+SKEW
            for h in range(HPG):
                kv = h // (HPG // KPG)
                ps_a = psAt.tile([128, 512], F32, tag="psAt", name="psAt_t")
                den = bigden.tile([128, 512], F32R, tag="den_acc", name="den_acc")
                pts = {}

                def consume(g):
                    pt = pts.pop(g)
                    t0, t1 = 2 * g, 2 * g + 1
                    off0 = max(0, 128 * (t0 - 4 * j))
                    off1 = max(0, 128 * (t1 - 4 * j))
                    # attn MMs first: they gate the PE stream
                    for tt, t, off in ((0, t0, off0), (1, t1, off1)):
                        nc.tensor.matmul(
                            ps_a[:, off:512],
                            v_sb[:, t * 256 + kv * 128 : t * 256 + kv * 128 + 128],
                            pt[:, tt * 512 + off : (tt + 1) * 512],
                            start=(t == 0),
                            stop=(t == n_sk - 1),
                        )
                    # denominator: cheap bf16 pair-sum, one f32 accumulation
                    first = t0 == 0
                    if off1 > off0:
                        if first:
                            nc.vector.tensor_copy(
                                den[:, off0:off1], pt[:, off0:off1]
                            )
                        else:
                            nc.vector.tensor_add(
                                den[:, off0:off1], den[:, off0:off1], pt[:, off0:off1]
                            )
                    pair = probpool.tile(
                        [128, 512], BF, tag="pair", name="pair", bufs=2
                    )
                    nc.vector.tensor_add(
                        pair[:, off1:512],
                        pt[:, off1:512],
                        pt[:, 512 + off1 : 1024],
                    )
                    if first:
                        nc.vector.tensor_copy(den[:, off1:512], pair[:, off1:512])
                    else:
                        nc.vector.tensor_add(
                            den[:, off1:512], den[:, off1:512], pair[:, off1:512]
                        )

                for g in range(n_grp):
                    ps_s = psS.tile([128, 1024], F32, tag="psS", name="psS_t")
                    for tt in range(2):
                        t = 2 * g + tt
                        nc.tensor.matmul(
                            ps_s[:, tt * 512 : (tt + 1) * 512],
                            kt[kv][:, t * 128 : (t + 1) * 128],
                            qt[h][:, sq0 : sq0 + 512],
                            start=True,
                            stop=True,
                        )
                    pt = probpool.tile([128, 1024], BF, tag="probs", name="probs_t")
                    nc.scalar.activation(pt[:], ps_s[:], EXP, scale=SCALE)
                    # zero the masked upper triangle of diagonal tiles (SBUF side)
                    for tt in range(2):
                        t = 2 * g + tt
                        r = t - 4 * j
                        if r >= 0:
                            lo = tt * 512 + 128 * r
                            nc.vector.tensor_mul(
                                pt[:, lo : lo + 128], pt[:, lo : lo + 128], tri_sb[:]
                            )
                    pts[g] = pt
                    if g >= SKEW:
                        consume(g - SKEW)
                for g in range(max(0, n_grp - SKEW), n_grp):
                    consume(g)

                ps_d = psGP.tile([128, 512], F32, tag="psGP", name="psD_t")
                nc.tensor.matmul(
                    ps_d[:], ones_sb[:], den[:], start=True, stop=True
                )
                inv_b = bigden.tile([128, 512], F32, tag="inv_b", name="inv_b")
                nc.vector.reciprocal(inv_b[:], ps_d[:])
                nc.vector.tensor_tensor(
                    at[h][:, sq0 : sq0 + 512], ps_a[:], inv_b[:], MULT
                )

        def oproj_all():
            for d in range(NOD):
                wod = wopool.tile([128, HPG * 512], BF, tag="wot", name="wod")
                nc.sync.dma_start(wod[:], wo[d])
                for m in range(NTT):
                    ps = psGP.tile([128, 512], F32, tag="psGP", name="ps_o")
                    for h in range(HPG):
                        nc.tensor.matmul(
                            ps[:],
                            at[h][:, m * 128 : (m + 1) * 128],
                            wod[:, h * 512 : (h + 1) * 512],
                            start=(h == 0),
                            stop=(h == HPG - 1),
                        )
                    osb = outpool.tile([128, 512], F32, tag="osb", name="osb")
                    nc.vector.tensor_copy(osb[:], ps[:])
                    nc.sync.dma_start(
                        outp[m * 128 : (m + 1) * 128, d * 512 : (d + 1) * 512], osb[:]
                    )

        def proj_panel0(n):
            # panel 0: emit the (large) V-weight DMA after the panel's own
            # x/w loads so the first Q/K chains start as early as possible;
            # V chains only need v_w_sb at the panel's tail.
            tok0 = n * PTOK
            halves = []
            for q4 in range(4):
                xq = xpool.tile([128, 8 * PTOK], BF, tag="xts", name="xq")
                nc.sync.dma_start(
                    xq.rearrange("p (f t) -> p f t", t=PTOK),
                    xt_v[:, q4 * 8 : (q4 + 1) * 8, tok0 : tok0 + PTOK],
                )
                halves.append((xq, q4 * 8))
            nc.sync.dma_start(v_w_sb[:], wv[:])
            return halves

        for _rep in range(reps):
            if do1:
                proj_panel(0, pre=proj_panel0)
            for n in range(1, NPANEL):
                if do1:
                    proj_panel(n)
                if do2:
                    attn_block(n - 1)
            if do2:
                attn_block(NPANEL - 1)
            if do3:
                oproj_all()

    nc.compile()
    return nc


_SPLIT_PERM = np.concatenate([np.arange(0, HD, 2), np.arange(1, HD, 2)])


def _host_prep(x, freqs_cos, freqs_sin, mask, wq, wk, wv, wo):
    """Build per-core input maps (8 cores = 2 batches x 4 head groups)."""
    x = np.asarray(x, np.float32)
    wq = np.asarray(wq, np.float32)
    wk = np.asarray(wk, np.float32)
    wv = np.asarray(wv, np.float32)
    wo = np.asarray(wo, np.float32)
    freqs_cos = np.asarray(freqs_cos, np.float32)
    freqs_sin = np.asarray(freqs_sin, np.float32)
    mask = np.asarray(mask, np.float32)

    xts = [np.ascontiguousarray(x[b].T).astype(BF16) for b in range(B)]

    ct = freqs_cos.T  # [64, S]
    st = freqs_sin.T
    cosb = np.concatenate([ct, ct], axis=0).astype(BF16)
    sinb = np.concatenate([-st, st], axis=0).astype(BF16)
    # trim[k_local, q_local] = 1 where k_local <= q_local (causal keep), else 0
    trim = np.ascontiguousarray(
        np.tril(np.ones((128, 128), dtype=np.float32)).T
    ).astype(BF16)
    ones32 = np.ones((128, 128), np.float32)

    per_g = []
    for g in range(G):
        wq_g = wq[:, g * HPG * HD : (g + 1) * HPG * HD].reshape(D, HPG, HD)
        wq_g = wq_g[:, :, _SPLIT_PERM]
        wq_g = np.ascontiguousarray(
            wq_g.reshape(NFT, 128, HPG, HD).transpose(2, 1, 0, 3).reshape(HPG, 128, NFT * 128)
        ).astype(BF16)

        wk_g = wk[:, g * KPG * HD : (g + 1) * KPG * HD].reshape(D, KPG, HD)
        wk_g = wk_g[:, :, _SPLIT_PERM]
        wk_g = np.ascontiguousarray(
            wk_g.reshape(NFT, 128, KPG, HD).transpose(2, 1, 0, 3).reshape(KPG, 128, NFT * 128)
        ).astype(BF16)

        wv_g = np.ascontiguousarray(
            wv[:, g * KPG * HD : (g + 1) * KPG * HD]
            .reshape(NFT, 128, KPG * 128)
            .transpose(1, 0, 2)
            .reshape(128, NFT * KPG * 128)
        ).astype(BF16)

        wo_g = wo[g * HPG * HD : (g + 1) * HPG * HD, :]
        wo_g = np.ascontiguousarray(
            wo_g.reshape(HPG, 128, NOD, 512).transpose(2, 1, 0, 3).reshape(NOD, 128, HPG * 512)
        ).astype(BF16)

        per_g.append((wq_g, wk_g, wv_g, wo_g))

    in_maps = []
    for core in range(8):
        b, g = divmod(core, G)
        wq_g, wk_g, wv_g, wo_g = per_g[g]
        in_maps.append(
            {
                "xt": xts[b],
                "wq": wq_g,
                "wk": wk_g,
                "wv": wv_g,
                "wo": wo_g,
                "cosb": cosb,
                "sinb": sinb,
                "trim": trim,
                "ones32": ones32,
            }
        )
    return in_maps


def get_program(phases=(1, 2, 3), reps=1):
    key = ("nc", tuple(phases), reps)
    if key not in _CACHE:
        _CACHE[key] = _build_program(phases, reps)
    return _CACHE[key]


def kernel(
    x, start_pos, freqs_cos, freqs_sin, mask, wq, wk, wv, wo, **_ignored
):
    nc = get_program()
    in_maps = _host_prep(x, freqs_cos, freqs_sin, mask, wq, wk, wv, wo)
    res = run_bass_kernel_spmd(nc, in_maps, core_ids=list(range(8)))
    partials = [res.results[c]["outp"] for c in range(8)]
    out = np.stack(
        [
            partials[b * G]
            + partials[b * G + 1]
            + partials[b * G + 2]
            + partials[b * G + 3]
            for b in range(B)
        ]
    ).astype(np.float32)
    return out

